# revision 15
# baseline (speedup 1.0000x reference)
"""FBP reconstructor on 8 TRN2 NeuronCores (Bass/Tile).

Pipeline (per core, angle-sharded):
  1. column sums of x with H-boundary masks  (PE matmul, colmask const)
  2. 9-tap 1D conv -> proj_sim^T [det, (b,a)] (PE matmul)
  3. Ram-Lak filter as two DFT matmuls with constant cos/sin matrices;
     ramlak applied on device as per-partition scalar multiply
  4. per-octet angle-table extraction via per-core selector matmuls
  5. interleaved segment table TAB[k] = (T[k], E0..E3[k]) where
     E0 = D[k], Er = D[k+r]-D[k+r-1], D[k] = T[k+1]-T[k] (D=0 past end)
  6. segment backprojection gather: one int16 anchor per 4 consecutive
     pixels (|slope| <= 1 so a d=5 window covers all taps), ap_gather
     d=5; exact linear interp via relu telescoping:
       val = T[a] + sum_r relu(pfrac - r) * Er[a]
     with host-precomputed fp16 pfrac = ix - anchor per pixel
  7. PE selector-matmul accumulation into PSUM, evict once per chunk

Host: shards angle flip-sym pairs across cores, builds constants,
segment anchors and fp16 pfrac tiles, merges the 8 partial outputs
(mirror-add for the flip-sym half) and scales.
"""

import sys

if "/opt/trn_rl_repo" not in sys.path:
    sys.path.insert(0, "/opt/trn_rl_repo")

import numpy as np

IMG = 256
DET = 256
NA = 180
NB = 4
NFREQ = DET // 2 + 1  # 129
SEG = 4  # pixels per gather index
DW = SEG + 2  # gather window (d); bf16 needs d*2 % 4 == 0, slot SEG+1 unused
CHUNK = 2048
NSEG = CHUNK // SEG  # 512 segments per chunk
NCHUNKS = IMG * IMG // CHUNK  # 32
NPAIR = NA // 2  # 90 flip-sym base pairs
NCORES = 8
BA = NB * NA  # 720
MT = 120  # ba M-tile size (6 tiles)

_cache = {}


def _host_constants():
    """Input-independent constants."""
    colmask = np.ones((IMG, 3), dtype=np.float32)
    colmask[IMG - 1, 0] = 0.0
    colmask[0, 2] = 0.0

    n = np.arange(DET)[:, None]
    k = np.arange(NFREQ)[None, :]
    phase = 2.0 * np.pi * n * k / DET
    dftAc = np.cos(phase).astype(np.float32)  # [256,129]
    dftAs = np.sin(phase).astype(np.float32)
    u = np.full(NFREQ, 2.0 / DET, dtype=np.float64)
    u[0] = 1.0 / DET
    u[NFREQ - 1] = 1.0 / DET
    m = np.arange(DET)[None, :]
    kk = np.arange(NFREQ)[:, None]
    phB = 2.0 * np.pi * kk * m / DET
    dftBc = (u[:, None] * np.cos(phB)).astype(np.float32)  # [129,256]
    dftBs = (u[:, None] * np.sin(phB)).astype(np.float32)

    # accumulation selectors [3][128, 8]: row 16g+c -> col c (c<8),
    # filtered by group parity for the two sweep-2 variants.
    sela = np.zeros((3, 128, 8), dtype=np.float32)
    for g in range(8):
        for c in range(8):
            sela[0, 16 * g + c, c] = 1.0
            if g % 2 == 0:
                sela[1, 16 * g + c, c] = 1.0
            else:
                sela[2, 16 * g + c, c] = 1.0

    # per-pair interp positions ix [NPAIR, IMG*IMG] (f32, matches ref)
    lin = np.linspace(-1.0, 1.0, IMG).astype(np.float32)
    yy, xx = np.meshgrid(lin, lin, indexing="ij")
    ang = np.linspace(0.0, np.pi, NA).astype(np.float32)
    ix_all = np.empty((NPAIR, IMG * IMG), dtype=np.float32)
    for i in range(NPAIR):
        c = np.float32(np.cos(ang[i]))
        s = np.float32(np.sin(ang[i]))
        t = (c * xx + s * yy).astype(np.float32)
        ix = np.clip(
            (t + np.float32(1.0)) * np.float32(0.5) * np.float32(DET - 1),
            0.0, DET - 1).astype(np.float32)
        ix_all[i] = ix.reshape(-1)
    return colmask, dftAc, dftAs, dftBc, dftBs, sela, ix_all


def _wrap16(flat_idx):
    """[NSEG] seg idx -> [16, NSEG//16] wrapped (i at partition i%16, col i//16)."""
    return flat_idx.reshape(NSEG // 16, 16).T


def _anchors_pfrac(ix):
    """ix [CHUNK] f32 -> (anchors [NSEG] int16, pfrac [CHUNK] f16)."""
    seg = ix.reshape(NSEG, SEG)
    anch = np.floor(seg.min(axis=1)).astype(np.int32)
    pf = (seg.astype(np.float64) - anch[:, None]).astype(np.float16)
    return anch.astype(np.int16), pf.reshape(-1)


def _per_core_inputs(ix_all):
    """Per-core SEL matrices, wrapped anchor tiles and fp16 pfrac tiles."""
    out = []
    for r in range(NCORES):
        pairs = list(range(r, NPAIR, NCORES))  # 11 or 12
        p1 = pairs[:8]
        p2 = pairs[8:]
        while len(p2) < 4:
            p2.append(-1)  # dummy slot

        sel1 = np.zeros((BA, 128), dtype=np.float32)
        for g, pi in enumerate(p1):
            for s in range(2):
                a = pi if s == 0 else NA - 1 - pi
                for b in range(NB):
                    sel1[b * NA + a, 16 * g + 4 * s + b] = 1.0
        sel2 = np.zeros((BA, 128), dtype=np.float32)
        for p, pi in enumerate(p2):
            if pi < 0:
                continue
            for h in range(2):
                g = 2 * p + h
                for s in range(2):
                    a = pi if s == 0 else NA - 1 - pi
                    for b in range(NB):
                        sel2[b * NA + a, 16 * g + 4 * s + b] = 1.0

        # precompute anchors/pfrac per (pair, chunk)
        apf = {}
        for pi in set(p1) | set(p2):
            if pi < 0:
                continue
            for ci in range(NCHUNKS):
                apf[(pi, ci)] = _anchors_pfrac(
                    ix_all[pi, ci * CHUNK:(ci + 1) * CHUNK])

        idx1 = np.zeros((NCHUNKS, 128, NSEG // 16), dtype=np.int16)
        pf1 = np.zeros((NCHUNKS, 128, CHUNK), dtype=np.float16)
        for ci in range(NCHUNKS):
            for g, pi in enumerate(p1):
                anch, pf = apf[(pi, ci)]
                idx1[ci, 16 * g:16 * g + 16, :] = _wrap16(anch)
                pf1[ci, 16 * g:16 * g + 16, :] = pf[None, :]
        idx2 = np.zeros((16, 128, NSEG // 16), dtype=np.int16)
        pf2 = np.zeros((16, 128, CHUNK), dtype=np.float16)
        for cc in range(16):
            for p, pi in enumerate(p2):
                if pi < 0:
                    continue
                for h in range(2):
                    ci = 2 * cc + h
                    g = 2 * p + h
                    anch, pf = apf[(pi, ci)]
                    idx2[cc, 16 * g:16 * g + 16, :] = _wrap16(anch)
                    pf2[cc, 16 * g:16 * g + 16, :] = pf[None, :]
        out.append((sel1, sel2, idx1, idx2,
                    pf1.reshape(NCHUNKS, 128, NSEG, SEG),
                    pf2.reshape(16, 128, NSEG, SEG)))
    return out


def _build_nc():
    import concourse.mybir as mybir
    import concourse.tile as tile
    from concourse import bacc
    from concourse.bass import broadcast_tensor_aps

    f32 = mybir.dt.float32
    f16 = mybir.dt.float16
    bf16 = mybir.dt.bfloat16
    i16 = mybir.dt.int16
    Act = mybir.ActivationFunctionType

    nc = bacc.Bacc(None, target_bir_lowering=False, debug=False)

    x_d = nc.dram_tensor("x3", [NB, IMG, IMG], f32, kind="ExternalInput")
    w9_d = nc.dram_tensor("w9", [3, 3, NA], f32, kind="ExternalInput")
    rcol_d = nc.dram_tensor("rcol", [128, 1], f32, kind="ExternalInput")
    rnyq_d = nc.dram_tensor("rnyq", [1, 1], f32, kind="ExternalInput")
    cm_d = nc.dram_tensor("colmask", [IMG, 3], f32, kind="ExternalInput")
    dAc_d = nc.dram_tensor("dftAc", [DET, NFREQ], f32, kind="ExternalInput")
    dAs_d = nc.dram_tensor("dftAs", [DET, NFREQ], f32, kind="ExternalInput")
    dBc_d = nc.dram_tensor("dftBc", [NFREQ, DET], f32, kind="ExternalInput")
    dBs_d = nc.dram_tensor("dftBs", [NFREQ, DET], f32, kind="ExternalInput")
    sela_d = nc.dram_tensor("sela", [128, 3, 8], f32, kind="ExternalInput")
    sel1_d = nc.dram_tensor("sel1", [BA, 128], f32, kind="ExternalInput")
    sel2_d = nc.dram_tensor("sel2", [BA, 128], f32, kind="ExternalInput")
    idx1_d = nc.dram_tensor("idx1", [NCHUNKS, 128, NSEG // 16], i16,
                            kind="ExternalInput")
    idx2_d = nc.dram_tensor("idx2", [16, 128, NSEG // 16], i16,
                            kind="ExternalInput")
    pf1_d = nc.dram_tensor("pf1", [NCHUNKS, 128, NSEG, SEG], f16,
                           kind="ExternalInput")
    pf2_d = nc.dram_tensor("pf2", [16, 128, NSEG, SEG], f16,
                           kind="ExternalInput")
    out_d = nc.dram_tensor("partial", [NCHUNKS, 8, CHUNK], f32,
                           kind="ExternalOutput")

    with tile.TileContext(nc) as tc:
        with (
            tc.tile_pool(name="persist", bufs=1) as pp,
            tc.tile_pool(name="tab", bufs=1) as tabp,
        ):
            # ---- persistent small tiles
            sela_t = pp.tile([128, 3, 8], f32)
            rcol_t = pp.tile([128, 1], f32)
            rnyq_t = pp.tile([1, 1], f32)
            nc.sync.dma_start(out=sela_t[:], in_=sela_d[:, :, :])
            selab_t = pp.tile([128, 3, 8], f16)
            nc.scalar.copy(out=selab_t[:], in_=sela_t[:])
            nc.sync.dma_start(out=rcol_t[:], in_=rcol_d[:, :])
            nc.sync.dma_start(out=rnyq_t[:], in_=rnyq_d[:, :])

            # interleaved segment tables per octet: (T, E0..E3)
            TAB_t = tabp.tile([128, 2, DET, DW], f16)
            # per-partition bias columns for relu taps: bias_t[:, r] = -r
            bias_t = pp.tile([128, SEG], f32)
            for r in range(SEG):
                nc.vector.memset(bias_t[:, r:r + 1], float(-r))

            with tc.tile_pool(name="setup", bufs=1) as sp:
                w9_t = sp.tile([3, 3, NA], f32)
                nc.sync.dma_start(out=w9_t[:], in_=w9_d[:, :, :])
                cm_t = sp.tile([128, 2, 3], f32)
                nc.sync.dma_start(out=cm_t[:, 0], in_=cm_d[0:128, :])
                nc.sync.dma_start(out=cm_t[:, 1], in_=cm_d[128:256, :])
                dAc_t = sp.tile([128, 2, NFREQ], f32)
                dAs_t = sp.tile([128, 2, NFREQ], f32)
                dBc_t = sp.tile([128, DET], f32)
                dBcn_t = sp.tile([1, DET], f32)
                dBs_t = sp.tile([128, DET], f32)
                dBsn_t = sp.tile([1, DET], f32)
                sel1_t = sp.tile([MT, 6, 128], f32)
                sel2_t = sp.tile([MT, 6, 128], f32)
                nc.sync.dma_start(out=dAc_t[:, 0], in_=dAc_d[0:128, :])
                nc.sync.dma_start(out=dAc_t[:, 1], in_=dAc_d[128:256, :])
                nc.sync.dma_start(out=dAs_t[:, 0], in_=dAs_d[0:128, :])
                nc.sync.dma_start(out=dAs_t[:, 1], in_=dAs_d[128:256, :])
                nc.sync.dma_start(out=dBc_t[:], in_=dBc_d[0:128, :])
                nc.sync.dma_start(out=dBcn_t[:], in_=dBc_d[128:129, :])
                nc.sync.dma_start(out=dBs_t[:], in_=dBs_d[0:128, :])
                nc.sync.dma_start(out=dBsn_t[:], in_=dBs_d[128:129, :])
                for t in range(6):
                    nc.sync.dma_start(out=sel1_t[:, t],
                                      in_=sel1_d[MT * t:MT * (t + 1), :])
                    nc.sync.dma_start(out=sel2_t[:, t],
                                      in_=sel2_d[MT * t:MT * (t + 1), :])

                pT_sb = sp.tile([128, 2, BA], f32)  # proj_sim^T [det, (b,a)]

                with (
                    tc.tile_pool(name="xload", bufs=2) as xp,
                    tc.tile_pool(name="ps_a", bufs=2, space="PSUM") as psa,
                ):
                    for b in range(NB):
                        xt = xp.tile([128, 2, IMG], f32)
                        nc.sync.dma_start(out=xt[:, 0], in_=x_d[b, 0:128, :])
                        nc.sync.dma_start(out=xt[:, 1], in_=x_d[b, 128:256, :])
                        tk_ps = psa.tile([3, IMG], f32)
                        nc.tensor.matmul(tk_ps[:], cm_t[:, 0], xt[:, 0],
                                         start=True, stop=False)
                        nc.tensor.matmul(tk_ps[:], cm_t[:, 1], xt[:, 1],
                                         start=False, stop=True)
                        # zero-padded column sums: tk3[_, 1+v] = Tk[v]
                        tk3_t = xp.tile([3, IMG + 2], f32)
                        nc.vector.memset(tk3_t[:], 0.0)
                        nc.scalar.copy(out=tk3_t[:, 1:IMG + 1], in_=tk_ps[:])
                        for jt in range(2):
                            pc_ps = psa.tile([128, NA], f32)
                            for kj in range(3):
                                nc.tensor.matmul(
                                    pc_ps[:],
                                    tk3_t[:, jt * 128 + kj:jt * 128 + kj + 128],
                                    w9_t[:, kj],
                                    start=(kj == 0), stop=(kj == 2))
                            nc.scalar.copy(
                                out=pT_sb[:, jt, b * NA:(b + 1) * NA],
                                in_=pc_ps[:])

                # ---- stage 1: spectra [freq, ba], ramlak-scaled
                spc_sb = sp.tile([128, BA], f32)
                sps_sb = sp.tile([128, BA], f32)
                spcn_sb = sp.tile([1, BA], f32)
                spsn_sb = sp.tile([1, BA], f32)
                with tc.tile_pool(name="ps_b", bufs=2, space="PSUM") as psb:
                    for var, dA_t, dst, dstn in (
                        (0, dAc_t, spc_sb, spcn_sb),
                        (1, dAs_t, sps_sb, spsn_sb),
                    ):
                        sp_ps = psb.tile([128, 1024], f32)
                        spn_ps = psb.tile([1, 1024], f32)
                        for ns in (slice(0, 512), slice(512, BA)):
                            for kt in range(2):
                                nc.tensor.matmul(
                                    sp_ps[:, ns], dA_t[:, kt, 0:128],
                                    pT_sb[:, kt, ns],
                                    start=(kt == 0), stop=(kt == 1))
                                nc.tensor.matmul(
                                    spn_ps[:, ns], dA_t[:, kt, 128:129],
                                    pT_sb[:, kt, ns],
                                    start=(kt == 0), stop=(kt == 1))
                        nc.vector.tensor_scalar_mul(dst[:], sp_ps[:, 0:BA],
                                                    rcol_t[:, 0:1])
                        nc.vector.tensor_scalar_mul(dstn[:], spn_ps[:, 0:BA],
                                                    rnyq_t[:, 0:1])

                # ---- stage 2: filtered [ba-tiles, det]
                filt_sb = sp.tile([MT, 6, DET], f32)
                with tc.tile_pool(name="ps_c", bufs=2, space="PSUM") as psc:
                    for mt in range(6):
                        ms = slice(mt * MT, (mt + 1) * MT)
                        f_ps = psc.tile([MT, DET], f32)
                        nc.tensor.matmul(f_ps[:], spc_sb[:, ms], dBc_t[:],
                                         start=True, stop=False)
                        nc.tensor.matmul(f_ps[:], spcn_sb[:, ms], dBcn_t[:],
                                         start=False, stop=False)
                        nc.tensor.matmul(f_ps[:], sps_sb[:, ms], dBs_t[:],
                                         start=False, stop=False)
                        nc.tensor.matmul(f_ps[:], spsn_sb[:, ms], dBsn_t[:],
                                         start=False, stop=True)
                        nc.scalar.copy(out=filt_sb[:, mt], in_=f_ps[:])

                # ---- per-octet tables T -> TAB (T, E0..E3)
                with tc.tile_pool(name="ps_d", bufs=2, space="PSUM") as psd:
                    for o, sel_t in ((1, sel2_t), (0, sel1_t)):
                        t_ps = psd.tile([128, DET], f32)
                        for kt in range(6):
                            nc.tensor.matmul(t_ps[:], sel_t[:, kt, :],
                                             filt_sb[:, kt, :],
                                             start=(kt == 0), stop=(kt == 5))
                        T_sb = sp.tile([128, DET], f32, tag="tsb")
                        nc.scalar.copy(out=T_sb[:], in_=t_ps[:])
                        # padded first differences Dp[k] = T[k+1]-T[k],
                        # zero for k >= DET-1 (and the DW-1 pad slots)
                        Dp_sb = sp.tile([128, DET + DW - 1], f32, tag="dpsb")
                        nc.vector.memset(Dp_sb[:], 0.0)
                        nc.vector.tensor_sub(Dp_sb[:, 0:DET - 1],
                                             T_sb[:, 1:DET],
                                             T_sb[:, 0:DET - 1])
                        # TAB[:, o, k, 0] = T[k]
                        nc.scalar.copy(out=TAB_t[:, o, :, 0], in_=T_sb[:])
                        # TAB[:, o, k, 1] = E0[k] = Dp[k]
                        nc.scalar.copy(out=TAB_t[:, o, :, 1],
                                       in_=Dp_sb[:, 0:DET])
                        # TAB[:, o, k, 1+r] = Dp[k+r] - Dp[k+r-1], r=1..3
                        for r in range(1, SEG):
                            nc.vector.tensor_sub(TAB_t[:, o, :, 1 + r],
                                                 Dp_sb[:, r:DET + r],
                                                 Dp_sb[:, r - 1:DET + r - 1])

            # ---- gather + extract + accumulate
            with (
                tc.tile_pool(name="idx", bufs=4) as idxp,
                tc.tile_pool(name="pfp", bufs=2) as pfp,
                tc.tile_pool(name="sg", bufs=2) as sgp,
                tc.tile_pool(name="extv", bufs=2) as extp,
                tc.tile_pool(name="exts", bufs=2) as extsp,
                tc.tile_pool(name="gout", bufs=2) as gop,
                tc.tile_pool(name="ps_acc", bufs=1, space="PSUM") as psacc,
            ):
                def extract(sg, pf, tag):
                    """val[c, j] = T[a] + sum_r relu(pf - r) * Er[a]."""
                    tsl = extp.tile([128, NSEG, 1], f16, tag=f"tsl{tag}")
                    val = extp.tile([128, NSEG, SEG], f16, tag=f"val{tag}")
                    tmp = extsp.tile([128, NSEG, SEG], f16, tag="tmp")
                    R4 = extsp.tile([128, SEG, NSEG, SEG], f16, tag="R4")
                    for r in range(SEG):
                        nc.scalar.activation(R4[:, r], pf[:], Act.Relu,
                                             bias=bias_t[:, r:r + 1])
                    nc.sync.dma_start(out=tsl[:], in_=sg[:, :, 0:1])
                    for r in range(SEG):
                        dst = val if r == 0 else tmp
                        ra, ea = broadcast_tensor_aps(
                            R4[:, r], sg[:, :, 1 + r:2 + r])
                        nc.vector.tensor_mul(dst[:], ra, ea)
                        if r > 0:
                            nc.vector.tensor_add(val[:], val[:], tmp[:])
                    return val, tsl

                for cc in range(16):
                    it2 = idxp.tile([128, NSEG // 16], i16, tag="it2")
                    nc.sync.dma_start(out=it2[:], in_=idx2_d[cc])
                    pf2_t = pfp.tile([128, NSEG, SEG], f16, tag="pf2")
                    nc.sync.dma_start(out=pf2_t[:], in_=pf2_d[cc])
                    sg2 = sgp.tile([128, NSEG, DW], f16, tag="sg2")
                    nc.gpsimd.ap_gather(sg2[:], TAB_t[:, 1], it2[:],
                                        channels=128, num_elems=DET,
                                        d=DW, num_idxs=NSEG)
                    val2, tsl2 = extract(sg2, pf2_t, "2")
                    for h in range(2):
                        ci = 2 * cc + h
                        it = idxp.tile([128, NSEG // 16], i16, tag="it1")
                        nc.sync.dma_start(out=it[:], in_=idx1_d[ci])
                        pf1_t = pfp.tile([128, NSEG, SEG], f16, tag="pf1")
                        nc.sync.dma_start(out=pf1_t[:], in_=pf1_d[ci])
                        sg1 = sgp.tile([128, NSEG, DW], f16, tag="sg1")
                        nc.gpsimd.ap_gather(sg1[:], TAB_t[:, 0], it[:],
                                            channels=128, num_elems=DET,
                                            d=DW, num_idxs=NSEG)
                        val1, tsl1 = extract(sg1, pf1_t, "1")
                        acc = psacc.tile([8, CHUNK], f32)
                        # per-region [start, stop] pairs adjacent (interleaved
                        # groups in a shared PSUM bank drop updates)
                        for j in range(4):
                            js = slice(128 * j, 128 * (j + 1))
                            _, t1b = broadcast_tensor_aps(
                                val1[:, js, :], tsl1[:, js, :])
                            _, t2b = broadcast_tensor_aps(
                                val2[:, js, :], tsl2[:, js, :])
                            ps = acc[:, 512 * j:512 * (j + 1)]
                            nc.tensor.matmul(ps, selab_t[:, 0, :],
                                             val1[:, js, :],
                                             start=True, stop=False)
                            nc.tensor.matmul(ps, selab_t[:, 0, :], t1b,
                                             start=False, stop=False)
                            nc.tensor.matmul(ps, selab_t[:, 1 + h, :],
                                             val2[:, js, :],
                                             start=False, stop=False)
                            nc.tensor.matmul(ps, selab_t[:, 1 + h, :], t2b,
                                             start=False, stop=True)
                        ev = gop.tile([8, CHUNK], f32, tag="ev")
                        nc.scalar.copy(out=ev[:], in_=acc[:])
                        nc.sync.dma_start(out=out_d[ci], in_=ev[:])
    nc.compile()
    return nc


def _get_compiled():
    if "nc" not in _cache:
        (colmask, dftAc, dftAs, dftBc, dftBs, sela, ix_all) = _host_constants()
        _cache["consts"] = (colmask, dftAc, dftAs, dftBc, dftBs, sela)
        _cache["percore"] = _per_core_inputs(ix_all)
        _cache["nc"] = _build_nc()
    return _cache["nc"], _cache["consts"], _cache["percore"]


def _in_maps(x, conv_w, ramlak, consts, percore):
    colmask, dftAc, dftAs, dftBc, dftBs, sela = consts
    x3 = np.ascontiguousarray(
        np.asarray(x, dtype=np.float32).reshape(NB, IMG, IMG))
    # w9[ki, kj, a] = conv_w[a, 0, ki, kj]; device tile partition axis = ki
    w9 = np.ascontiguousarray(
        np.asarray(conv_w, dtype=np.float32).reshape(NA, 3, 3).transpose(1, 2, 0))
    r = np.asarray(ramlak, dtype=np.float32)
    common = {
        "x3": x3, "w9": w9,
        "rcol": np.ascontiguousarray(r[0:128].reshape(128, 1)),
        "rnyq": np.ascontiguousarray(r[128:129].reshape(1, 1)),
        "colmask": colmask, "dftAc": dftAc, "dftAs": dftAs,
        "dftBc": dftBc, "dftBs": dftBs,
        "sela": np.ascontiguousarray(sela.transpose(1, 0, 2)),
    }
    in_maps = []
    for r_ in range(NCORES):
        sel1, sel2, idx1, idx2, pf1, pf2 = percore[r_]
        m = dict(common)
        m.update({"sel1": sel1, "sel2": sel2, "idx1": idx1, "idx2": idx2,
                  "pf1": pf1, "pf2": pf2})
        in_maps.append(m)
    return in_maps


def kernel(x, conv_w, ramlak):
    from concourse.bass_utils import run_bass_kernel_spmd

    nc, consts, percore = _get_compiled()
    in_maps = _in_maps(x, conv_w, ramlak, consts, percore)
    res = run_bass_kernel_spmd(nc, in_maps, list(range(NCORES)))

    total = np.zeros((8, IMG * IMG), dtype=np.float32)
    for r_ in range(NCORES):
        part = res.results[r_]["partial"]  # [32 ci, 8 m, 2048 u]
        img = np.transpose(part, (1, 0, 2)).reshape(8, IMG * IMG)
        total += img
    direct = total[0:4].reshape(NB, IMG, IMG)
    mirrored = total[4:8].reshape(NB, IMG, IMG)[:, :, ::-1]
    out = (direct + mirrored) * np.float32(np.pi / NA)
    return np.ascontiguousarray(out.reshape(NB, 1, IMG, IMG)).astype(np.float32)


# revision 16
# speedup vs baseline: 11.7125x; 11.7125x over previous
"""FBP reconstructor on 8 TRN2 NeuronCores (Bass/Tile).

Pipeline (per core, angle-sharded):
  1. column sums of x with H-boundary masks  (PE matmul, colmask const)
  2. 9-tap 1D conv -> proj_sim^T [det, (b,a)] (PE matmul)
  3. Ram-Lak filter as two DFT matmuls with constant cos/sin matrices;
     ramlak applied on device as per-partition scalar multiply
  4. per-octet angle-table extraction via per-core selector matmuls
  5. interleaved segment table TAB[k] = (T[k], E0..E3[k]) where
     E0 = D[k], Er = D[k+r]-D[k+r-1], D[k] = T[k+1]-T[k] (D=0 past end)
  6. segment backprojection gather: one int16 anchor per 4 consecutive
     pixels (|slope| <= 1 so a d=5 window covers all taps), ap_gather
     d=5; exact linear interp via relu telescoping:
       val = T[a] + sum_r relu(pfrac - r) * Er[a]
     with host-precomputed fp16 pfrac = ix - anchor per pixel
  7. PE selector-matmul accumulation into PSUM, evict once per chunk

Host: shards angle flip-sym pairs across cores, builds constants,
segment anchors and fp16 pfrac tiles, merges the 8 partial outputs
(mirror-add for the flip-sym half) and scales.
"""

import sys

if "/opt/trn_rl_repo" not in sys.path:
    sys.path.insert(0, "/opt/trn_rl_repo")

import numpy as np

IMG = 256
DET = 256
NA = 180
NB = 4
NFREQ = DET // 2 + 1  # 129
SEG = 4  # pixels per gather index
DW = SEG + 2  # gather window (d); bf16 needs d*2 % 4 == 0, slot SEG+1 unused
CHUNK = 2048
NSEG = CHUNK // SEG  # 512 segments per chunk
NCHUNKS = IMG * IMG // CHUNK  # 32
NPAIR = NA // 2  # 90 flip-sym base pairs
NCORES = 8
BA = NB * NA  # 720
MT = 120  # ba M-tile size (6 tiles)

_cache = {}


def _host_constants():
    """Input-independent constants."""
    colmask = np.ones((IMG, 3), dtype=np.float32)
    colmask[IMG - 1, 0] = 0.0
    colmask[0, 2] = 0.0

    n = np.arange(DET)[:, None]
    k = np.arange(NFREQ)[None, :]
    phase = 2.0 * np.pi * n * k / DET
    dftAc = np.cos(phase).astype(np.float32)  # [256,129]
    dftAs = np.sin(phase).astype(np.float32)
    u = np.full(NFREQ, 2.0 / DET, dtype=np.float64)
    u[0] = 1.0 / DET
    u[NFREQ - 1] = 1.0 / DET
    m = np.arange(DET)[None, :]
    kk = np.arange(NFREQ)[:, None]
    phB = 2.0 * np.pi * kk * m / DET
    dftBc = (u[:, None] * np.cos(phB)).astype(np.float32)  # [129,256]
    dftBs = (u[:, None] * np.sin(phB)).astype(np.float32)

    # accumulation selectors [3][128, 8]: row 16g+c -> col c (c<8),
    # filtered by group parity for the two sweep-2 variants.
    sela = np.zeros((3, 128, 8), dtype=np.float32)
    for g in range(8):
        for c in range(8):
            sela[0, 16 * g + c, c] = 1.0
            if g % 2 == 0:
                sela[1, 16 * g + c, c] = 1.0
            else:
                sela[2, 16 * g + c, c] = 1.0

    # per-pair interp positions ix [NPAIR, IMG*IMG] (f32, matches ref)
    lin = np.linspace(-1.0, 1.0, IMG).astype(np.float32)
    yy, xx = np.meshgrid(lin, lin, indexing="ij")
    ang = np.linspace(0.0, np.pi, NA).astype(np.float32)
    ix_all = np.empty((NPAIR, IMG * IMG), dtype=np.float32)
    for i in range(NPAIR):
        c = np.float32(np.cos(ang[i]))
        s = np.float32(np.sin(ang[i]))
        t = (c * xx + s * yy).astype(np.float32)
        ix = np.clip(
            (t + np.float32(1.0)) * np.float32(0.5) * np.float32(DET - 1),
            0.0, DET - 1).astype(np.float32)
        ix_all[i] = ix.reshape(-1)
    return colmask, dftAc, dftAs, dftBc, dftBs, sela, ix_all


def _wrap16(flat_idx):
    """[NSEG] seg idx -> [16, NSEG//16] wrapped (i at partition i%16, col i//16)."""
    return flat_idx.reshape(NSEG // 16, 16).T


def _anchors_pfrac(ix):
    """ix [CHUNK] f32 -> (anchors [NSEG] int16, pfrac [CHUNK] f16)."""
    seg = ix.reshape(NSEG, SEG)
    anch = np.floor(seg.min(axis=1)).astype(np.int32)
    pf = (seg.astype(np.float64) - anch[:, None]).astype(np.float16)
    return anch.astype(np.int16), pf.reshape(-1)


def _per_core_inputs(ix_all):
    """Per-core SEL matrices, wrapped anchor tiles and fp16 pfrac tiles."""
    out = []
    for r in range(NCORES):
        pairs = list(range(r, NPAIR, NCORES))  # 11 or 12
        p1 = pairs[:8]
        p2 = pairs[8:]
        while len(p2) < 4:
            p2.append(-1)  # dummy slot

        sel1 = np.zeros((BA, 128), dtype=np.float32)
        for g, pi in enumerate(p1):
            for s in range(2):
                a = pi if s == 0 else NA - 1 - pi
                for b in range(NB):
                    sel1[b * NA + a, 16 * g + 4 * s + b] = 1.0
        sel2 = np.zeros((BA, 128), dtype=np.float32)
        for p, pi in enumerate(p2):
            if pi < 0:
                continue
            for h in range(2):
                g = 2 * p + h
                for s in range(2):
                    a = pi if s == 0 else NA - 1 - pi
                    for b in range(NB):
                        sel2[b * NA + a, 16 * g + 4 * s + b] = 1.0

        # precompute anchors/pfrac per (pair, chunk)
        apf = {}
        for pi in set(p1) | set(p2):
            if pi < 0:
                continue
            for ci in range(NCHUNKS):
                apf[(pi, ci)] = _anchors_pfrac(
                    ix_all[pi, ci * CHUNK:(ci + 1) * CHUNK])

        idx1 = np.zeros((NCHUNKS, 128, NSEG // 16), dtype=np.int16)
        pf1 = np.zeros((NCHUNKS, 128, CHUNK), dtype=np.float16)
        for ci in range(NCHUNKS):
            for g, pi in enumerate(p1):
                anch, pf = apf[(pi, ci)]
                idx1[ci, 16 * g:16 * g + 16, :] = _wrap16(anch)
                pf1[ci, 16 * g:16 * g + 16, :] = pf[None, :]
        idx2 = np.zeros((16, 128, NSEG // 16), dtype=np.int16)
        pf2 = np.zeros((16, 128, CHUNK), dtype=np.float16)
        for cc in range(16):
            for p, pi in enumerate(p2):
                if pi < 0:
                    continue
                for h in range(2):
                    ci = 2 * cc + h
                    g = 2 * p + h
                    anch, pf = apf[(pi, ci)]
                    idx2[cc, 16 * g:16 * g + 16, :] = _wrap16(anch)
                    pf2[cc, 16 * g:16 * g + 16, :] = pf[None, :]
        out.append((sel1, sel2, idx1, idx2,
                    pf1.reshape(NCHUNKS, 128, NSEG, SEG),
                    pf2.reshape(16, 128, NSEG, SEG)))
    return out


def _build_nc():
    import concourse.mybir as mybir
    import concourse.tile as tile
    from concourse import bacc
    from concourse.bass import broadcast_tensor_aps

    f32 = mybir.dt.float32
    f16 = mybir.dt.float16
    bf16 = mybir.dt.bfloat16
    i16 = mybir.dt.int16
    Act = mybir.ActivationFunctionType

    nc = bacc.Bacc(None, target_bir_lowering=False, debug=False)

    x_d = nc.dram_tensor("x3", [NB, IMG, IMG], f32, kind="ExternalInput")
    w9_d = nc.dram_tensor("w9", [3, 3, NA], f32, kind="ExternalInput")
    rcol_d = nc.dram_tensor("rcol", [128, 1], f32, kind="ExternalInput")
    rnyq_d = nc.dram_tensor("rnyq", [1, 1], f32, kind="ExternalInput")
    cm_d = nc.dram_tensor("colmask", [IMG, 3], f32, kind="ExternalInput")
    dAc_d = nc.dram_tensor("dftAc", [DET, NFREQ], f32, kind="ExternalInput")
    dAs_d = nc.dram_tensor("dftAs", [DET, NFREQ], f32, kind="ExternalInput")
    dBc_d = nc.dram_tensor("dftBc", [NFREQ, DET], f32, kind="ExternalInput")
    dBs_d = nc.dram_tensor("dftBs", [NFREQ, DET], f32, kind="ExternalInput")
    sela_d = nc.dram_tensor("sela", [128, 3, 8], f32, kind="ExternalInput")
    sel1_d = nc.dram_tensor("sel1", [BA, 128], f32, kind="ExternalInput")
    sel2_d = nc.dram_tensor("sel2", [BA, 128], f32, kind="ExternalInput")
    idx1_d = nc.dram_tensor("idx1", [NCHUNKS, 128, NSEG // 16], i16,
                            kind="ExternalInput")
    idx2_d = nc.dram_tensor("idx2", [16, 128, NSEG // 16], i16,
                            kind="ExternalInput")
    pf1_d = nc.dram_tensor("pf1", [NCHUNKS, 128, NSEG, SEG], f16,
                           kind="ExternalInput")
    pf2_d = nc.dram_tensor("pf2", [16, 128, NSEG, SEG], f16,
                           kind="ExternalInput")
    out_d = nc.dram_tensor("partial", [NCHUNKS, 8, CHUNK], f32,
                           kind="ExternalOutput")

    with tile.TileContext(nc) as tc:
        with (
            tc.tile_pool(name="persist", bufs=1) as pp,
            tc.tile_pool(name="tab", bufs=1) as tabp,
        ):
            # ---- persistent small tiles
            sela_t = pp.tile([128, 3, 8], f32)
            rcol_t = pp.tile([128, 1], f32)
            rnyq_t = pp.tile([1, 1], f32)
            nc.sync.dma_start(out=sela_t[:], in_=sela_d[:, :, :])
            selab_t = pp.tile([128, 3, 8], f16)
            nc.scalar.copy(out=selab_t[:], in_=sela_t[:])
            nc.sync.dma_start(out=rcol_t[:], in_=rcol_d[:, :])
            nc.sync.dma_start(out=rnyq_t[:], in_=rnyq_d[:, :])

            # interleaved segment tables per octet: (T, E0..E3)
            TAB_t = tabp.tile([128, 2, DET, DW], f16)
            # per-partition bias columns for relu taps: bias_t[:, r] = -r
            bias_t = pp.tile([128, SEG], f32)
            for r in range(SEG):
                nc.vector.memset(bias_t[:, r:r + 1], float(-r))

            with tc.tile_pool(name="setup", bufs=1) as sp:
                w9_t = sp.tile([3, 3, NA], f32)
                nc.sync.dma_start(out=w9_t[:], in_=w9_d[:, :, :])
                cm_t = sp.tile([128, 2, 3], f32)
                nc.sync.dma_start(out=cm_t[:, 0], in_=cm_d[0:128, :])
                nc.sync.dma_start(out=cm_t[:, 1], in_=cm_d[128:256, :])
                dAc_t = sp.tile([128, 2, NFREQ], f32)
                dAs_t = sp.tile([128, 2, NFREQ], f32)
                dBc_t = sp.tile([128, DET], f32)
                dBcn_t = sp.tile([1, DET], f32)
                dBs_t = sp.tile([128, DET], f32)
                dBsn_t = sp.tile([1, DET], f32)
                sel1_t = sp.tile([MT, 6, 128], f32)
                sel2_t = sp.tile([MT, 6, 128], f32)
                nc.sync.dma_start(out=dAc_t[:, 0], in_=dAc_d[0:128, :])
                nc.sync.dma_start(out=dAc_t[:, 1], in_=dAc_d[128:256, :])
                nc.sync.dma_start(out=dAs_t[:, 0], in_=dAs_d[0:128, :])
                nc.sync.dma_start(out=dAs_t[:, 1], in_=dAs_d[128:256, :])
                nc.sync.dma_start(out=dBc_t[:], in_=dBc_d[0:128, :])
                nc.sync.dma_start(out=dBcn_t[:], in_=dBc_d[128:129, :])
                nc.sync.dma_start(out=dBs_t[:], in_=dBs_d[0:128, :])
                nc.sync.dma_start(out=dBsn_t[:], in_=dBs_d[128:129, :])
                for t in range(6):
                    nc.sync.dma_start(out=sel1_t[:, t],
                                      in_=sel1_d[MT * t:MT * (t + 1), :])
                    nc.sync.dma_start(out=sel2_t[:, t],
                                      in_=sel2_d[MT * t:MT * (t + 1), :])

                pT_sb = sp.tile([128, 2, BA], f32)  # proj_sim^T [det, (b,a)]

                with (
                    tc.tile_pool(name="xload", bufs=2) as xp,
                    tc.tile_pool(name="ps_a", bufs=2, space="PSUM") as psa,
                ):
                    for b in range(NB):
                        xt = xp.tile([128, 2, IMG], f32)
                        nc.sync.dma_start(out=xt[:, 0], in_=x_d[b, 0:128, :])
                        nc.sync.dma_start(out=xt[:, 1], in_=x_d[b, 128:256, :])
                        tk_ps = psa.tile([3, IMG], f32)
                        nc.tensor.matmul(tk_ps[:], cm_t[:, 0], xt[:, 0],
                                         start=True, stop=False)
                        nc.tensor.matmul(tk_ps[:], cm_t[:, 1], xt[:, 1],
                                         start=False, stop=True)
                        # zero-padded column sums: tk3[_, 1+v] = Tk[v]
                        tk3_t = xp.tile([3, IMG + 2], f32)
                        nc.vector.memset(tk3_t[:], 0.0)
                        nc.scalar.copy(out=tk3_t[:, 1:IMG + 1], in_=tk_ps[:])
                        for jt in range(2):
                            pc_ps = psa.tile([128, NA], f32)
                            for kj in range(3):
                                nc.tensor.matmul(
                                    pc_ps[:],
                                    tk3_t[:, jt * 128 + kj:jt * 128 + kj + 128],
                                    w9_t[:, kj],
                                    start=(kj == 0), stop=(kj == 2))
                            nc.scalar.copy(
                                out=pT_sb[:, jt, b * NA:(b + 1) * NA],
                                in_=pc_ps[:])

                # ---- stage 1: spectra [freq, ba], ramlak-scaled
                spc_sb = sp.tile([128, BA], f32)
                sps_sb = sp.tile([128, BA], f32)
                spcn_sb = sp.tile([1, BA], f32)
                spsn_sb = sp.tile([1, BA], f32)
                with tc.tile_pool(name="ps_b", bufs=2, space="PSUM") as psb:
                    for var, dA_t, dst, dstn in (
                        (0, dAc_t, spc_sb, spcn_sb),
                        (1, dAs_t, sps_sb, spsn_sb),
                    ):
                        sp_ps = psb.tile([128, 1024], f32)
                        spn_ps = psb.tile([1, 1024], f32)
                        for ns in (slice(0, 512), slice(512, BA)):
                            for kt in range(2):
                                nc.tensor.matmul(
                                    sp_ps[:, ns], dA_t[:, kt, 0:128],
                                    pT_sb[:, kt, ns],
                                    start=(kt == 0), stop=(kt == 1))
                                nc.tensor.matmul(
                                    spn_ps[:, ns], dA_t[:, kt, 128:129],
                                    pT_sb[:, kt, ns],
                                    start=(kt == 0), stop=(kt == 1))
                        nc.vector.tensor_scalar_mul(dst[:], sp_ps[:, 0:BA],
                                                    rcol_t[:, 0:1])
                        nc.vector.tensor_scalar_mul(dstn[:], spn_ps[:, 0:BA],
                                                    rnyq_t[:, 0:1])

                # ---- stage 2: filtered [ba-tiles, det]
                filt_sb = sp.tile([MT, 6, DET], f32)
                with tc.tile_pool(name="ps_c", bufs=2, space="PSUM") as psc:
                    for mt in range(6):
                        ms = slice(mt * MT, (mt + 1) * MT)
                        f_ps = psc.tile([MT, DET], f32)
                        nc.tensor.matmul(f_ps[:], spc_sb[:, ms], dBc_t[:],
                                         start=True, stop=False)
                        nc.tensor.matmul(f_ps[:], spcn_sb[:, ms], dBcn_t[:],
                                         start=False, stop=False)
                        nc.tensor.matmul(f_ps[:], sps_sb[:, ms], dBs_t[:],
                                         start=False, stop=False)
                        nc.tensor.matmul(f_ps[:], spsn_sb[:, ms], dBsn_t[:],
                                         start=False, stop=True)
                        nc.scalar.copy(out=filt_sb[:, mt], in_=f_ps[:])

                # ---- per-octet tables T -> TAB (T, E0..E3)
                with tc.tile_pool(name="ps_d", bufs=2, space="PSUM") as psd:
                    for o, sel_t in ((1, sel2_t), (0, sel1_t)):
                        t_ps = psd.tile([128, DET], f32)
                        for kt in range(6):
                            nc.tensor.matmul(t_ps[:], sel_t[:, kt, :],
                                             filt_sb[:, kt, :],
                                             start=(kt == 0), stop=(kt == 5))
                        T_sb = sp.tile([128, DET], f32, tag="tsb")
                        nc.scalar.copy(out=T_sb[:], in_=t_ps[:])
                        # padded first differences Dp[k] = T[k+1]-T[k],
                        # zero for k >= DET-1 (and the DW-1 pad slots)
                        Dp_sb = sp.tile([128, DET + DW - 1], f32, tag="dpsb")
                        nc.vector.memset(Dp_sb[:], 0.0)
                        nc.vector.tensor_sub(Dp_sb[:, 0:DET - 1],
                                             T_sb[:, 1:DET],
                                             T_sb[:, 0:DET - 1])
                        # TAB[:, o, k, 0] = T[k]
                        nc.scalar.copy(out=TAB_t[:, o, :, 0], in_=T_sb[:])
                        # TAB[:, o, k, 1] = E0[k] = Dp[k]
                        nc.scalar.copy(out=TAB_t[:, o, :, 1],
                                       in_=Dp_sb[:, 0:DET])
                        # TAB[:, o, k, 1+r] = Dp[k+r] - Dp[k+r-1], r=1..3
                        for r in range(1, SEG):
                            nc.vector.tensor_sub(TAB_t[:, o, :, 1 + r],
                                                 Dp_sb[:, r:DET + r],
                                                 Dp_sb[:, r - 1:DET + r - 1])

            # ---- gather + extract + accumulate
            with (
                tc.tile_pool(name="idx", bufs=4) as idxp,
                tc.tile_pool(name="pfp", bufs=2) as pfp,
                tc.tile_pool(name="sg", bufs=2) as sgp,
                tc.tile_pool(name="extv", bufs=2) as extp,
                tc.tile_pool(name="exts", bufs=2) as extsp,
                tc.tile_pool(name="gout", bufs=2) as gop,
                tc.tile_pool(name="ps_acc", bufs=1, space="PSUM") as psacc,
            ):
                def extract(sg, pf, tag):
                    """val[c, j] = T[a] + sum_r relu(pf - r) * Er[a]."""
                    val = extp.tile([128, NSEG, SEG], f16, tag=f"val{tag}")
                    tmp = extsp.tile([128, NSEG, SEG], f16, tag="tmp")
                    R4 = extsp.tile([128, SEG, NSEG, SEG], f16, tag="R4")
                    for r in range(SEG):
                        nc.scalar.activation(R4[:, r], pf[:], Act.Relu,
                                             bias=bias_t[:, r:r + 1])
                    for r in range(SEG):
                        dst = val if r == 0 else tmp
                        ra, ea = broadcast_tensor_aps(
                            R4[:, r], sg[:, :, 1 + r:2 + r])
                        nc.vector.tensor_mul(dst[:], ra, ea)
                        if r > 0:
                            nc.vector.tensor_add(val[:], val[:], tmp[:])
                    return val

                for cc in range(16):
                    it2 = idxp.tile([128, NSEG // 16], i16, tag="it2")
                    nc.sync.dma_start(out=it2[:], in_=idx2_d[cc])
                    pf2_t = pfp.tile([128, NSEG, SEG], f16, tag="pf2")
                    nc.sync.dma_start(out=pf2_t[:], in_=pf2_d[cc])
                    sg2 = sgp.tile([128, NSEG, DW], f16, tag="sg2")
                    nc.gpsimd.ap_gather(sg2[:], TAB_t[:, 1], it2[:],
                                        channels=128, num_elems=DET,
                                        d=DW, num_idxs=NSEG)
                    val2 = extract(sg2, pf2_t, "2")
                    for h in range(2):
                        ci = 2 * cc + h
                        it = idxp.tile([128, NSEG // 16], i16, tag="it1")
                        nc.sync.dma_start(out=it[:], in_=idx1_d[ci])
                        pf1_t = pfp.tile([128, NSEG, SEG], f16, tag="pf1")
                        nc.sync.dma_start(out=pf1_t[:], in_=pf1_d[ci])
                        sg1 = sgp.tile([128, NSEG, DW], f16, tag="sg1")
                        nc.gpsimd.ap_gather(sg1[:], TAB_t[:, 0], it[:],
                                            channels=128, num_elems=DET,
                                            d=DW, num_idxs=NSEG)
                        val1 = extract(sg1, pf1_t, "1")
                        acc = psacc.tile([8, CHUNK], f32)
                        # per-region [start, stop] pairs adjacent (interleaved
                        # groups in a shared PSUM bank drop updates)
                        for j in range(4):
                            js = slice(128 * j, 128 * (j + 1))
                            _, t1b = broadcast_tensor_aps(
                                val1[:, js, :], sg1[:, js, 0:1])
                            _, t2b = broadcast_tensor_aps(
                                val2[:, js, :], sg2[:, js, 0:1])
                            ps = acc[:, 512 * j:512 * (j + 1)]
                            nc.tensor.matmul(ps, selab_t[:, 0, :],
                                             val1[:, js, :],
                                             start=True, stop=False)
                            nc.tensor.matmul(ps, selab_t[:, 0, :], t1b,
                                             start=False, stop=False)
                            nc.tensor.matmul(ps, selab_t[:, 1 + h, :],
                                             val2[:, js, :],
                                             start=False, stop=False)
                            nc.tensor.matmul(ps, selab_t[:, 1 + h, :], t2b,
                                             start=False, stop=True)
                        ev = gop.tile([8, CHUNK], f32, tag="ev")
                        nc.scalar.copy(out=ev[:], in_=acc[:])
                        nc.sync.dma_start(out=out_d[ci], in_=ev[:])
    nc.compile()
    return nc


def _get_compiled():
    if "nc" not in _cache:
        (colmask, dftAc, dftAs, dftBc, dftBs, sela, ix_all) = _host_constants()
        _cache["consts"] = (colmask, dftAc, dftAs, dftBc, dftBs, sela)
        _cache["percore"] = _per_core_inputs(ix_all)
        _cache["nc"] = _build_nc()
    return _cache["nc"], _cache["consts"], _cache["percore"]


def _in_maps(x, conv_w, ramlak, consts, percore):
    colmask, dftAc, dftAs, dftBc, dftBs, sela = consts
    x3 = np.ascontiguousarray(
        np.asarray(x, dtype=np.float32).reshape(NB, IMG, IMG))
    # w9[ki, kj, a] = conv_w[a, 0, ki, kj]; device tile partition axis = ki
    w9 = np.ascontiguousarray(
        np.asarray(conv_w, dtype=np.float32).reshape(NA, 3, 3).transpose(1, 2, 0))
    r = np.asarray(ramlak, dtype=np.float32)
    common = {
        "x3": x3, "w9": w9,
        "rcol": np.ascontiguousarray(r[0:128].reshape(128, 1)),
        "rnyq": np.ascontiguousarray(r[128:129].reshape(1, 1)),
        "colmask": colmask, "dftAc": dftAc, "dftAs": dftAs,
        "dftBc": dftBc, "dftBs": dftBs,
        "sela": np.ascontiguousarray(sela.transpose(1, 0, 2)),
    }
    in_maps = []
    for r_ in range(NCORES):
        sel1, sel2, idx1, idx2, pf1, pf2 = percore[r_]
        m = dict(common)
        m.update({"sel1": sel1, "sel2": sel2, "idx1": idx1, "idx2": idx2,
                  "pf1": pf1, "pf2": pf2})
        in_maps.append(m)
    return in_maps


def kernel(x, conv_w, ramlak):
    from concourse.bass_utils import run_bass_kernel_spmd

    nc, consts, percore = _get_compiled()
    in_maps = _in_maps(x, conv_w, ramlak, consts, percore)
    res = run_bass_kernel_spmd(nc, in_maps, list(range(NCORES)))

    total = np.zeros((8, IMG * IMG), dtype=np.float32)
    for r_ in range(NCORES):
        part = res.results[r_]["partial"]  # [32 ci, 8 m, 2048 u]
        img = np.transpose(part, (1, 0, 2)).reshape(8, IMG * IMG)
        total += img
    direct = total[0:4].reshape(NB, IMG, IMG)
    mirrored = total[4:8].reshape(NB, IMG, IMG)[:, :, ::-1]
    out = (direct + mirrored) * np.float32(np.pi / NA)
    return np.ascontiguousarray(out.reshape(NB, 1, IMG, IMG)).astype(np.float32)
